# revision 53
# baseline (speedup 1.0000x reference)
"""Trainium2 Bass kernel for dual-branch local+dilated windowed attention.

Problem: B=1, L=4096, D=512, H=8 heads (dh=64), window=+-256, dilation=4.
reference returns (out_local, out_dilated), each [1, L, D] fp32.

Sharding: sequence (L) sharded across 8 cores; each core owns 512 query rows
and loads a 1024-row key slice (256-row halo each side, zero-padded at the
sequence edges).  All weights are replicated, pre-transposed, pre-shuffled to
the on-chip partition-major layout host-side (so every DMA is per-partition
contiguous with large descriptors), and cast to bf16 (wq/wk to fp8) with the
rmsnorm gains and the 1/sqrt(dh) score scale folded in.

v2 structure (vs the v1 baseline at ~131us):
  - DMA: x and weights pre-shuffled host-side to [128, ...] partition-major
    so descriptors are 2-4KB contiguous; spread across the sync/scalar/gpsimd
    queues in first-needed order; x split in 4 chunks so rmsnorm starts early.
  - attention pipelined over 16 (branch, head) units with scores emitted one
    unit ahead, 3 score-psum banks to decouple PE from ACT exp, and the
    dilated branch's projections emitted as PE filler work between local
    units (ACT exp is the per-unit bottleneck for the local branch).
  - per-head AV accumulates all 4 query tiles into one PSUM bank so the
    softmax denominators batch into a single reciprocal + one broadcast mul.
  - AV output transposes are head-paired: two heads' [128,64] normalized
    context slabs transpose as one [128,128] PE op straight into the OT
    layout that the (row-parallel) Wo matmul wants.
  - PSUM->SBUF copies balanced across DVE/ACT/GPSIMD by phase load.
"""

import numpy as np
import ml_dtypes

L, D, H, DH = 4096, 512, 8, 64
WIN, DIL = 256, 4
EPS = 1e-6
NCORES = 8
QL = L // NCORES          # 512 queries per core
KL = QL + 2 * WIN         # 1024 keys per core (halo)
P = 128
NKC = KL // P             # 8 key chunks
NQT = QL // P             # 4 query tiles
BF16 = ml_dtypes.bfloat16
FP8 = ml_dtypes.float8_e4m3fn

_STATE = {}


def _build_nc():
    import concourse.bacc as bacc
    import concourse.tile as tile
    import concourse.mybir as mybir
    from concourse.masks import make_identity
    from concourse.bass import broadcast_tensor_aps, AP

    f32 = mybir.dt.float32
    bf16 = mybir.dt.bfloat16
    fp8 = mybir.dt.float8e4
    Exp = mybir.ActivationFunctionType.Exp
    Square = mybir.ActivationFunctionType.Square
    Sqrt = mybir.ActivationFunctionType.Sqrt
    DR = mybir.MatmulPerfMode.DoubleRow
    Mult = mybir.AluOpType.mult

    nc = bacc.Bacc()

    xn = nc.dram_tensor("xn", [P, NKC, D], bf16, kind="ExternalInput")
    wT = {}
    for br in ("l", "d"):
        for w in ("wq", "wk", "wv", "wo"):
            dt_w = fp8 if w in ("wq", "wk") else bf16
            wT[w, br] = nc.dram_tensor(f"{w}T_{br}", [P, 4, D], dt_w,
                                       kind="ExternalInput")
    tri_lo_d = nc.dram_tensor("tri_lo", [P, P], bf16, kind="ExternalInput")
    tri_hi_d = nc.dram_tensor("tri_hi", [P, P], bf16, kind="ExternalInput")
    colmask_d_ = {
        "l": nc.dram_tensor("colmask_l", [P, NKC], f32, kind="ExternalInput"),
        "d": nc.dram_tensor("colmask_d", [P, NKC], f32, kind="ExternalInput"),
    }
    out_dram = {
        "l": nc.dram_tensor("out_l", [QL, D], bf16, kind="ExternalOutput"),
        "d": nc.dram_tensor("out_d", [QL, D], bf16, kind="ExternalOutput"),
    }

    with tile.TileContext(nc) as tc:
        with (
            tc.tile_pool(name="singles", bufs=1) as singles,
            tc.tile_pool(name="xpool", bufs=3) as xpool,
            tc.tile_pool(name="small", bufs=8) as small,
            tc.tile_pool(name="expp", bufs=4) as expp,
            tc.tile_pool(name="osbp", bufs=2) as osbp,
            tc.tile_pool(name="outp", bufs=2) as outp,
            tc.tile_pool(name="pst", bufs=3, space="PSUM") as p_st,
            tc.tile_pool(name="pop", bufs=2, space="PSUM") as p_op,
            tc.tile_pool(name="ppr", bufs=2, space="PSUM") as p_pr,
            tc.tile_pool(name="ptr", bufs=1, space="PSUM") as p_tr,
        ):
            identity = singles.tile([P, P], bf16)
            make_identity(nc, identity)
            eps_t = singles.tile([P, 1], f32, name="eps")
            nc.vector.memset(eps_t, EPS)

            # ---- input + weight DMAs: partition-major contiguous layouts,
            # spread over the two HWDGE queues + SWDGE, first-needed first.
            # query-range tiles (tt 2-5) first so the Q projection can start
            # before the halo tiles land
            xsb = singles.tile([P, NKC, D], bf16, name="xsb")
            for eng, t0, ntt in ((nc.sync, 2, 1), (nc.scalar, 3, 1),
                                 (nc.sync, 4, 2), (nc.scalar, 0, 2),
                                 (nc.sync, 6, 2)):
                eng.dma_start(xsb[:, t0:t0 + ntt, :], xn[:, t0:t0 + ntt, :])
            w_sb = {}
            for w, br in [("wq", "l"), ("wk", "l"), ("wv", "l"), ("wo", "l"),
                          ("wq", "d"), ("wk", "d"), ("wv", "d"), ("wo", "d")]:
                w_sb[w, br] = singles.tile(
                    [P, 4, D], fp8 if w in ("wq", "wk") else bf16,
                    name=f"{w}_{br}")
            for w, br in (("wq", "l"), ("wk", "l"), ("wv", "l"), ("wo", "l")):
                nc.sync.dma_start(w_sb[w, br], wT[w, br][:, :, :])
            colmask = {}
            for br in ("l", "d"):
                colmask[br] = singles.tile([P, NKC], f32, name=f"cm_{br}")
                nc.gpsimd.dma_start(colmask[br], colmask_d_[br][:, :])
            tri_lo = singles.tile([P, P], bf16)
            nc.gpsimd.dma_start(tri_lo, tri_lo_d[:, :])
            tri_hi = singles.tile([P, P], bf16)
            nc.gpsimd.dma_start(tri_hi, tri_hi_d[:, :])

            xhatT = singles.tile([P, 4, KL], fp8, name="xhatT")
            xhatTb = singles.tile([P, 4, KL], bf16, name="xhatTb")



            QT, KT, V, OT = {}, {}, {}, {}
            for br in ("l", "d"):
                QT[br] = singles.tile([P, 4, QL], bf16, name=f"QT_{br}")
                KT[br] = singles.tile([P, 4, KL], bf16, name=f"KT_{br}")
                V[br] = singles.tile([P, NKC, H, DH + 1], bf16, name=f"V_{br}")
                OT[br] = singles.tile([P, 4, QL], bf16, name=f"OT_{br}")

            def key_cols_ap(ic, kc, br):
                # lhsT [128, 128] of xhat^T columns for key chunk kc
                if br == "l":
                    return xhatTb[:, ic, kc * P:(kc + 1) * P]
                rho, s = kc // 2, kc % 2
                return xhatTb[:, ic, :].rearrange(
                    "p (b four) -> p four b", four=DIL)[:, rho, s * P:(s + 1) * P]

            # ---- projection emitters (engine-parameterized copies) ----
            # NOTE: GPSIMD cannot read PSUM, so all PSUM->SBUF casts go on
            # DVE/ACT; GPSIMD takes the SBUF-only colmask multiplies.
            def _copy(eng, dst, src):
                if eng is nc.scalar:
                    nc.scalar.copy(dst, src)
                else:
                    eng.tensor_copy(dst, src)

            def emit_qproj(br, pair, ceng):
                ps = p_pr.tile([P, D], f32, tag="pp")
                for ic in (0, 2):
                    nc.tensor.matmul(
                        ps, w_sb["wq", br][:, ic:ic + 2, pair * P:(pair + 1) * P],
                        xhatT[:, ic:ic + 2, WIN:WIN + QL],
                        start=(ic == 0), stop=(ic == 2), perf_mode=DR)
                _copy(ceng, QT[br][:, pair, :], ps)

            def emit_kproj(br, pair, half, ceng):
                ps = p_pr.tile([P, D], f32, tag="pp")
                for ic in (0, 2):
                    nc.tensor.matmul(
                        ps, w_sb["wk", br][:, ic:ic + 2, pair * P:(pair + 1) * P],
                        xhatT[:, ic:ic + 2, half * D:(half + 1) * D],
                        start=(ic == 0), stop=(ic == 2), perf_mode=DR)
                _copy(ceng, KT[br][:, pair, half * D:(half + 1) * D], ps)

            def emit_vproj(br, kc, ceng):
                ps = p_pr.tile([P, D], f32, tag="pp")
                for ic in range(4):
                    nc.tensor.matmul(
                        ps, key_cols_ap(ic, kc, br),
                        w_sb["wv", br][:, ic, :],
                        start=(ic == 0), stop=(ic == 3))
                _copy(ceng, V[br][:, kc, :, 0:DH],
                      ps.rearrange("p (h dv) -> p h dv", h=H))
                nc.vector.memset(V[br][:, kc, :, DH:DH + 1], 1.0)
                nc.vector.tensor_scalar_mul(
                    V[br][:, kc], V[br][:, kc], colmask[br][:, kc:kc + 1])

            # ---- rmsnorm + transpose (x^2 sums alternate ACT/DVE) ----
            # query-range tiles first; the local Q projection is emitted as
            # soon as its four token tiles (tt 2-5) are transposed.
            for idx, tt in enumerate((2, 3, 4, 5, 0, 1, 6, 7)):
                xt = xsb[:, tt, :]
                sqd = xpool.tile([P, D], bf16, tag="sqd")
                ssum = small.tile([P, 1], f32, tag="ssum")
                if tt % 2 == 0:
                    nc.scalar.activation(sqd, xt, Square, accum_out=ssum)
                else:
                    nc.vector.scalar_tensor_tensor(
                        sqd, xt, 1.0, xt, Mult, Mult, accum_out=ssum)
                rstd = small.tile([P, 1], f32, tag="rstd")
                nc.scalar.activation(rstd, ssum, Sqrt, bias=eps_t, scale=1.0 / D)
                nc.vector.reciprocal(rstd, rstd)
                xh = xpool.tile([P, D], bf16, tag="xh")
                nc.vector.tensor_scalar_mul(xh, xt, rstd)
                tpb = p_pr.tile([P, D], bf16, tag="pp")
                for ic in range(4):
                    nc.tensor.transpose(tpb[:, ic * P:(ic + 1) * P],
                                        xh[:, ic * P:(ic + 1) * P], identity)
                nc.vector.tensor_copy(
                    xhatT[:, :, tt * P:(tt + 1) * P],
                    tpb.rearrange("p (ic q) -> p ic q", ic=4))
                nc.scalar.copy(
                    xhatTb[:, :, tt * P:(tt + 1) * P],
                    tpb.rearrange("p (ic q) -> p ic q", ic=4))
                if idx == 3:
                    for pair in range(4):
                        emit_qproj("l", pair, nc.vector)

            # dilated-branch weights are first needed ~half-way through the
            # local attention phase; the (otherwise idle) gpsimd SWDGE queue
            # issues them so neither HWDGE compute queue pays issue costs.
            for w, br in (("wq", "d"), ("wk", "d"), ("wv", "d"), ("wo", "d")):
                nc.gpsimd.dma_start(w_sb[w, br], wT[w, br][:, :, :])

            # ---- remaining local projections (DVE/ACT casts) ----
            for pair in range(4):
                for half in range(2):
                    emit_kproj("l", pair, half, nc.vector)
            for kc in range(NKC):
                emit_vproj("l", kc, nc.scalar)

            # dilated-branch projections become PE filler work interleaved
            # between local attention units (ACT exp binds there, PE has
            # slack); Q/K casts on DVE, V copies on ACT to split the load.
            fillers = []
            for pair in range(4):
                fillers.append(
                    lambda pair=pair: emit_qproj("d", pair, nc.vector))
            for pair in range(4):
                for half in range(2):
                    fillers.append(
                        lambda pair=pair, half=half:
                        emit_kproj("d", pair, half, nc.vector))
            for kc in range(NKC):
                fillers.append(lambda kc=kc: emit_vproj("d", kc, nc.vector))

            # ---- attention ----
            def _masks(br, ex):
                # edge triangle masks: chunk kc==qtile -> tri_lo at
                # q-offset 128*kc; chunk kc==qtile+4 -> tri_hi at
                # 128*(kc-4).  Both strides are uniform in the flat view,
                # so batch into 3 DVE ops instead of 8.
                C = QL if br == "l" else P
                stride = C + P
                hi0 = 4 * C if br == "l" else P
                exf = ex.rearrange("p a b -> p (a b)")
                g1 = exf[:, 0:4 * stride].rearrange(
                    "p (a c) -> p a c", c=stride)[:, :, 0:P]
                # the tri_hi group also has 4 blocks at a uniform stride, but
                # its rearrange view would run past the tensor end, so build
                # the [p, 4, P] access pattern directly (only in-bounds
                # columns are ever touched).
                base = exf[:, hi0:hi0 + P]
                g2 = AP(base.tensor, base.offset,
                        [list(base.ap[0]), [stride, 4], [1, P]])
                for g, tri in ((g1, tri_lo), (g2, tri_hi)):
                    ga, ta = broadcast_tensor_aps(
                        g, tri[:, :].rearrange("p (o b) -> p o b", o=1))
                    nc.vector.tensor_mul(ga, ga, ta)

            def scores_pair(br, pair):
                # the two heads of a pair live on SBUF partition halves
                # 0-63 / 64-127, i.e. row tiles T0/T8 of the 64x128 PE
                # tiling mode -- interleaving their score matmuls lets the
                # PE overlap one tile's LdWeights with the other's Matmul.
                if br == "l":
                    exs = [expp.tile([P, NKC, QL], bf16, tag="exp",
                                     name="ex_e"),
                           expp.tile([P, NKC, QL], bf16, tag="exp",
                                     name="ex_o")]
                    for kc in range(NKC):
                        qlo = max(0, P * (kc - 4))
                        qhi = min(QL, P * kc + P)
                        n = qhi - qlo
                        for sub in range(2):
                            r0 = 64 * sub
                            st = p_st.tile([P, QL], f32, tag="st")
                            nc.tensor.matmul(
                                st[:, :n],
                                KT[br][r0:r0 + 64, pair, kc * P:(kc + 1) * P],
                                QT[br][r0:r0 + 64, pair, qlo:qhi])
                            nc.scalar.activation(
                                exs[sub][:, kc, qlo:qhi], st[:, :n], Exp,
                                scale=1.0 / 32768)
                else:
                    exs = [expp.tile([P, NKC, P], bf16, tag="expd",
                                     name="ex_e"),
                           expp.tile([P, NKC, P], bf16, tag="expd",
                                     name="ex_o")]
                    for half in range(2):
                        for sub in range(2):
                            r0 = 64 * sub
                            st = p_st.tile([P, QL], f32, tag="st")
                            for j in range(4):
                                idx = half * 4 + j
                                rho, s = idx // 2, idx % 2
                                ktv = KT[br][r0:r0 + 64, pair, :].rearrange(
                                    "p (b four) -> p four b", four=DIL
                                )[:, rho, s * P:(s + 1) * P]
                                qtv = QT[br][r0:r0 + 64, pair, :].rearrange(
                                    "p (a four) -> p four a", four=DIL)[:, rho, :]
                                nc.tensor.matmul(st[:, j * P:(j + 1) * P],
                                                 ktv, qtv)
                            nc.scalar.activation(
                                exs[sub][:, half * 4:(half + 1) * 4, :], st,
                                Exp, scale=1.0 / 32768)
                _masks(br, exs[0])
                _masks(br, exs[1])
                return exs

            def av_head(br, h, ex, osb2):
                # all 4 qtiles of this head accumulate into one PSUM bank so
                # the denominators batch: one reciprocal + one broadcast mul.
                chunk_sets = ([(t, range(t, t + 5)) for t in range(NQT)]
                              if br == "l" else
                              [(rho, (rho * 2, rho * 2 + 1)) for rho in range(DIL)])
                op = p_op.tile([P, NQT, DH + 1], f32, tag="op")
                for t, kcs in chunk_sets:
                    kcs = list(kcs)
                    for r, kc in enumerate(kcs):
                        src = (ex[:, kc, t * P:(t + 1) * P] if br == "l"
                               else ex[:, kc, :])
                        nc.tensor.matmul(
                            op[:, t, :], src, V[br][:, kc, h, :],
                            start=(r == 0), stop=(r == len(kcs) - 1))
                rcp = small.tile([P, NQT], f32, tag="rcp")
                nc.vector.reciprocal(
                    rcp, op[:, :, DH:DH + 1].rearrange("p a o -> p (a o)"))
                dst = osb2[:, :, h % 2, :]
                num, rb = broadcast_tensor_aps(
                    op[:, :, 0:DH], rcp.rearrange("p (a o) -> p a o", o=1))
                nc.vector.tensor_mul(dst, num, rb)

            def pair_transpose(br, pair, osb2):
                # two heads' [128q, 64dv] slabs transpose as one [128,128] op
                tpb = p_tr.tile([P, NQT * P], bf16, tag="tpb")
                for t in range(NQT):
                    nc.tensor.transpose(
                        tpb[:, t * P:(t + 1) * P],
                        osb2[:, t].rearrange("p a b -> p (a b)"), identity)
                # d-branch OT copies run in the DVE-paced dilated window --
                # ACT idles there, so it takes them; the final pair's copy
                # gates the tail wo chain, so it goes to the then-idle DVE
                if br == "l" or pair == 3:
                    nc.vector.tensor_copy(OT[br][:, pair, :], tpb)
                else:
                    nc.scalar.copy(OT[br][:, pair, :], tpb)

            def wo_out(br, t):
                ps = p_pr.tile([P, D], f32, tag="pp")
                for pair in range(4):
                    nc.tensor.matmul(
                        ps, OT[br][:, pair, t * P:(t + 1) * P],
                        w_sb["wo", br][:, pair, :],
                        start=(pair == 0), stop=(pair == 3))
                ob = outp.tile([P, D], bf16, tag="ob")
                # local-branch obs land in the dilated window where ACT has
                # slack and DVE is the pacer; dilated obs land in the idle
                # tail where DVE is free
                if br == "l":
                    nc.scalar.copy(ob, ps)
                else:
                    nc.vector.tensor_copy(ob, ps)
                if br == "l":
                    nc.sync.dma_start(out_dram[br][t * P:(t + 1) * P, :], ob)
                else:
                    dst = out_dram[br][:, :].rearrange(
                        "(a four) o -> four a o", four=DIL)[t]
                    nc.scalar.dma_start(dst, ob)

            punits = [("l", p) for p in range(4)] + [("d", p) for p in range(4)]
            pexs = {0: scores_pair(*punits[0])}
            nfill = 0
            for i, (br, pr) in enumerate(punits):
                if i + 1 < len(punits):
                    pexs[i + 1] = scores_pair(*punits[i + 1])
                ex_e, ex_o = pexs.pop(i)
                osb2 = osbp.tile([P, NQT, 2, DH], bf16, tag="osb2",
                                 name="osb2")
                av_head(br, 2 * pr, ex_e, osb2)
                av_head(br, 2 * pr + 1, ex_o, osb2)
                pair_transpose(br, pr, osb2)
                if br == "l":
                    want = min(len(fillers), 5 * (i + 1))
                    while nfill < want:
                        fillers[nfill]()
                        nfill += 1
                if (br, pr) == ("l", 3):
                    while nfill < len(fillers):
                        fillers[nfill]()
                        nfill += 1
                    for t in range(NQT):
                        wo_out("l", t)
            for t in range(NQT):
                wo_out("d", t)

    nc.finalize()
    return nc


def _prep_host(x, key_padding_mask, weights):
    """Build the per-core input maps (weights shared across cores).

    All device tensors are pre-shuffled to the on-chip partition-major layout
    so every DMA moves per-partition-contiguous 2-4KB blocks.
    """
    x = np.asarray(x, dtype=np.float32).reshape(L, D)
    kpm = np.asarray(key_padding_mask).reshape(L).astype(bool)

    shared = {}
    for name, arr in weights.items():
        if name.startswith("wq") or name.startswith("wk"):
            pre = arr.reshape(4, P, D).transpose(1, 0, 2)
            shared[name] = np.ascontiguousarray(pre).astype(FP8)
        else:
            pre = arr.reshape(4, P, D).transpose(1, 0, 2)
            shared[name] = np.ascontiguousarray(pre).astype(BF16)

    idx = np.arange(P)
    tri_lo = (idx[:, None] >= idx[None, :]).astype(BF16)
    tri_hi = (idx[:, None] <= idx[None, :]).astype(BF16)
    shared["tri_lo"], shared["tri_hi"] = tri_lo, tri_hi

    valid_full = np.zeros(L + 2 * WIN, dtype=np.float32)
    valid_full[WIN:WIN + L] = (~kpm).astype(np.float32)

    in_maps = []
    for c in range(NCORES):
        lo = c * QL - WIN
        xnc = np.zeros((KL, D), dtype=np.float32)
        a, b = max(lo, 0), min(lo + KL, L)
        xnc[a - lo:b - lo] = x[a:b]
        v = valid_full[lo + WIN:lo + WIN + KL]  # validity of keys lo..lo+KL
        cm_l = v.reshape(NKC, P).T.astype(np.float32)
        # dilated chunk idx = rho*2+s holds keys lk = 4*(128*s + p) + rho
        cm_d = np.empty((P, NKC), dtype=np.float32)
        for rho in range(DIL):
            for s in range(2):
                lk = DIL * (P * s + idx) + rho
                cm_d[:, rho * 2 + s] = v[lk]
        m = dict(shared)
        m["xn"] = np.ascontiguousarray(
            xnc.reshape(NKC, P, D).transpose(1, 0, 2)).astype(BF16)
        m["colmask_l"] = np.ascontiguousarray(cm_l)
        m["colmask_d"] = np.ascontiguousarray(cm_d)
        in_maps.append(m)
    return in_maps


def kernel(x, key_padding_mask, wq_l, wk_l, wv_l, wo_l,
           wq_d, wk_d, wv_d, wo_d, g_q, g_kv, **run_kwargs):
    from concourse.bass_utils import run_bass_kernel_spmd

    g_q = np.asarray(g_q, dtype=np.float32)
    g_kv = np.asarray(g_kv, dtype=np.float32)
    scale = 1.0 / np.sqrt(DH)
    weights = {
        "wqT_l": np.asarray(wq_l, np.float32).T * (g_q * scale)[:, None] * 512.0,
        "wkT_l": np.asarray(wk_l, np.float32).T * g_kv[:, None] * 64.0,
        "wvT_l": np.asarray(wv_l, np.float32).T * g_kv[:, None],
        "woT_l": np.asarray(wo_l, np.float32).T,
        "wqT_d": np.asarray(wq_d, np.float32).T * (g_q * scale)[:, None] * 512.0,
        "wkT_d": np.asarray(wk_d, np.float32).T * g_kv[:, None] * 64.0,
        "wvT_d": np.asarray(wv_d, np.float32).T * g_kv[:, None],
        "woT_d": np.asarray(wo_d, np.float32).T,
    }
    in_maps = _prep_host(x, key_padding_mask, weights)

    if "nc" not in _STATE:
        _STATE["nc"] = _build_nc()
    res = run_bass_kernel_spmd(_STATE["nc"], in_maps,
                               core_ids=list(range(NCORES)), **run_kwargs)
    _STATE["last_result"] = res

    out_l = np.concatenate(
        [np.asarray(res.results[c]["out_l"], dtype=np.float32)
         for c in range(NCORES)], axis=0).reshape(1, L, D)
    out_d = np.concatenate(
        [np.asarray(res.results[c]["out_d"], dtype=np.float32)
         for c in range(NCORES)], axis=0).reshape(1, L, D)
    return (out_l, out_d)


# revision 57
# speedup vs baseline: 1.0050x; 1.0050x over previous
"""Trainium2 Bass kernel for dual-branch local+dilated windowed attention.

Problem: B=1, L=4096, D=512, H=8 heads (dh=64), window=+-256, dilation=4.
reference returns (out_local, out_dilated), each [1, L, D] fp32.

Sharding: sequence (L) sharded across 8 cores; each core owns 512 query rows
and loads a 1024-row key slice (256-row halo each side, zero-padded at the
sequence edges).  All weights are replicated, pre-transposed, pre-shuffled to
the on-chip partition-major layout host-side (so every DMA is per-partition
contiguous with large descriptors), and cast to bf16 (wq/wk to fp8) with the
rmsnorm gains and the 1/sqrt(dh) score scale folded in.

v2 structure (vs the v1 baseline at ~131us):
  - DMA: x and weights pre-shuffled host-side to [128, ...] partition-major
    so descriptors are 2-4KB contiguous; spread across the sync/scalar/gpsimd
    queues in first-needed order; x split in 4 chunks so rmsnorm starts early.
  - attention pipelined over 16 (branch, head) units with scores emitted one
    unit ahead, 3 score-psum banks to decouple PE from ACT exp, and the
    dilated branch's projections emitted as PE filler work between local
    units (ACT exp is the per-unit bottleneck for the local branch).
  - per-head AV accumulates all 4 query tiles into one PSUM bank so the
    softmax denominators batch into a single reciprocal + one broadcast mul.
  - AV output transposes are head-paired: two heads' [128,64] normalized
    context slabs transpose as one [128,128] PE op straight into the OT
    layout that the (row-parallel) Wo matmul wants.
  - PSUM->SBUF copies balanced across DVE/ACT/GPSIMD by phase load.
"""

import numpy as np
import ml_dtypes

L, D, H, DH = 4096, 512, 8, 64
WIN, DIL = 256, 4
EPS = 1e-6
NCORES = 8
QL = L // NCORES          # 512 queries per core
KL = QL + 2 * WIN         # 1024 keys per core (halo)
P = 128
NKC = KL // P             # 8 key chunks
NQT = QL // P             # 4 query tiles
BF16 = ml_dtypes.bfloat16
FP8 = ml_dtypes.float8_e4m3fn

_STATE = {}


def _build_nc():
    import concourse.bacc as bacc
    import concourse.tile as tile
    import concourse.mybir as mybir
    from concourse.masks import make_identity
    from concourse.bass import broadcast_tensor_aps, AP

    f32 = mybir.dt.float32
    bf16 = mybir.dt.bfloat16
    fp8 = mybir.dt.float8e4
    Exp = mybir.ActivationFunctionType.Exp
    Square = mybir.ActivationFunctionType.Square
    Sqrt = mybir.ActivationFunctionType.Sqrt
    DR = mybir.MatmulPerfMode.DoubleRow
    Mult = mybir.AluOpType.mult

    nc = bacc.Bacc()

    xn = nc.dram_tensor("xn", [P, NKC, D], bf16, kind="ExternalInput")
    wT = {}
    for br in ("l", "d"):
        for w in ("wq", "wk", "wv", "wo"):
            dt_w = fp8 if w in ("wq", "wk") else bf16
            wT[w, br] = nc.dram_tensor(f"{w}T_{br}", [P, 4, D], dt_w,
                                       kind="ExternalInput")
    tri_lo_d = nc.dram_tensor("tri_lo", [P, P], bf16, kind="ExternalInput")
    tri_hi_d = nc.dram_tensor("tri_hi", [P, P], bf16, kind="ExternalInput")
    colmask_d_ = {
        "l": nc.dram_tensor("colmask_l", [P, NKC], f32, kind="ExternalInput"),
        "d": nc.dram_tensor("colmask_d", [P, NKC], f32, kind="ExternalInput"),
    }
    out_dram = {
        "l": nc.dram_tensor("out_l", [QL, D], bf16, kind="ExternalOutput"),
        "d": nc.dram_tensor("out_d", [QL, D], bf16, kind="ExternalOutput"),
    }

    with tile.TileContext(nc) as tc:
        with (
            tc.tile_pool(name="singles", bufs=1) as singles,
            tc.tile_pool(name="xpool", bufs=3) as xpool,
            tc.tile_pool(name="small", bufs=8) as small,
            tc.tile_pool(name="expp", bufs=4) as expp,
            tc.tile_pool(name="osbp", bufs=2) as osbp,
            tc.tile_pool(name="outp", bufs=2) as outp,
            tc.tile_pool(name="pst", bufs=3, space="PSUM") as p_st,
            tc.tile_pool(name="pop", bufs=2, space="PSUM") as p_op,
            tc.tile_pool(name="ppr", bufs=2, space="PSUM") as p_pr,
            tc.tile_pool(name="ptr", bufs=1, space="PSUM") as p_tr,
        ):
            identity = singles.tile([P, P], bf16)
            make_identity(nc, identity)
            eps_t = singles.tile([P, 1], f32, name="eps")
            nc.vector.memset(eps_t, EPS)

            # ---- input + weight DMAs: partition-major contiguous layouts,
            # spread over the two HWDGE queues + SWDGE, first-needed first.
            # query-range tiles (tt 2-5) first so the Q projection can start
            # before the halo tiles land
            xsb = singles.tile([P, NKC, D], bf16, name="xsb")
            for eng, t0, ntt in ((nc.sync, 2, 2), (nc.scalar, 4, 2),
                                 (nc.sync, 0, 2), (nc.scalar, 6, 2)):
                eng.dma_start(xsb[:, t0:t0 + ntt, :], xn[:, t0:t0 + ntt, :])
            w_sb = {}
            for w, br in [("wq", "l"), ("wk", "l"), ("wv", "l"), ("wo", "l"),
                          ("wq", "d"), ("wk", "d"), ("wv", "d"), ("wo", "d")]:
                w_sb[w, br] = singles.tile(
                    [P, 4, D], fp8 if w in ("wq", "wk") else bf16,
                    name=f"{w}_{br}")
            for w, br in (("wq", "l"), ("wk", "l"), ("wv", "l"), ("wo", "l")):
                nc.sync.dma_start(w_sb[w, br], wT[w, br][:, :, :])
            colmask = {}
            for br in ("l", "d"):
                colmask[br] = singles.tile([P, NKC], f32, name=f"cm_{br}")
                nc.gpsimd.dma_start(colmask[br], colmask_d_[br][:, :])
            tri_lo = singles.tile([P, P], bf16)
            nc.gpsimd.dma_start(tri_lo, tri_lo_d[:, :])
            tri_hi = singles.tile([P, P], bf16)
            nc.gpsimd.dma_start(tri_hi, tri_hi_d[:, :])

            xhatT = singles.tile([P, 4, KL], fp8, name="xhatT")
            xhatTb = singles.tile([P, 4, KL], bf16, name="xhatTb")



            QT, KT, V, OT = {}, {}, {}, {}
            for br in ("l", "d"):
                QT[br] = singles.tile([P, 4, QL], bf16, name=f"QT_{br}")
                KT[br] = singles.tile([P, 4, KL], bf16, name=f"KT_{br}")
                V[br] = singles.tile([P, NKC, H, DH + 1], bf16, name=f"V_{br}")
                OT[br] = singles.tile([P, 4, QL], bf16, name=f"OT_{br}")

            def key_cols_ap(ic, kc, br):
                # lhsT [128, 128] of xhat^T columns for key chunk kc
                if br == "l":
                    return xhatTb[:, ic, kc * P:(kc + 1) * P]
                rho, s = kc // 2, kc % 2
                return xhatTb[:, ic, :].rearrange(
                    "p (b four) -> p four b", four=DIL)[:, rho, s * P:(s + 1) * P]

            # ---- projection emitters (engine-parameterized copies) ----
            # NOTE: GPSIMD cannot read PSUM, so all PSUM->SBUF casts go on
            # DVE/ACT; GPSIMD takes the SBUF-only colmask multiplies.
            def _copy(eng, dst, src):
                if eng is nc.scalar:
                    nc.scalar.copy(dst, src)
                else:
                    eng.tensor_copy(dst, src)

            def emit_qproj(br, pair, ceng):
                ps = p_pr.tile([P, D], f32, tag="pp")
                for ic in (0, 2):
                    nc.tensor.matmul(
                        ps, w_sb["wq", br][:, ic:ic + 2, pair * P:(pair + 1) * P],
                        xhatT[:, ic:ic + 2, WIN:WIN + QL],
                        start=(ic == 0), stop=(ic == 2), perf_mode=DR)
                _copy(ceng, QT[br][:, pair, :], ps)

            def emit_kproj(br, pair, half, ceng):
                ps = p_pr.tile([P, D], f32, tag="pp")
                for ic in (0, 2):
                    nc.tensor.matmul(
                        ps, w_sb["wk", br][:, ic:ic + 2, pair * P:(pair + 1) * P],
                        xhatT[:, ic:ic + 2, half * D:(half + 1) * D],
                        start=(ic == 0), stop=(ic == 2), perf_mode=DR)
                _copy(ceng, KT[br][:, pair, half * D:(half + 1) * D], ps)

            def emit_vproj(br, kc, ceng):
                ps = p_pr.tile([P, D], f32, tag="pp")
                for ic in range(4):
                    nc.tensor.matmul(
                        ps, key_cols_ap(ic, kc, br),
                        w_sb["wv", br][:, ic, :],
                        start=(ic == 0), stop=(ic == 3))
                _copy(ceng, V[br][:, kc, :, 0:DH],
                      ps.rearrange("p (h dv) -> p h dv", h=H))
                nc.vector.memset(V[br][:, kc, :, DH:DH + 1], 1.0)
                nc.vector.tensor_scalar_mul(
                    V[br][:, kc], V[br][:, kc], colmask[br][:, kc:kc + 1])

            # ---- rmsnorm + transpose (x^2 sums alternate ACT/DVE) ----
            # query-range tiles first; the local Q projection is emitted as
            # soon as its four token tiles (tt 2-5) are transposed.
            for idx, tt in enumerate((2, 3, 4, 5, 0, 1, 6, 7)):
                xt = xsb[:, tt, :]
                sqd = xpool.tile([P, D], bf16, tag="sqd")
                ssum = small.tile([P, 1], f32, tag="ssum")
                if tt % 2 == 0:
                    nc.scalar.activation(sqd, xt, Square, accum_out=ssum)
                else:
                    nc.vector.scalar_tensor_tensor(
                        sqd, xt, 1.0, xt, Mult, Mult, accum_out=ssum)
                rstd = small.tile([P, 1], f32, tag="rstd")
                nc.scalar.activation(rstd, ssum, Sqrt, bias=eps_t, scale=1.0 / D)
                nc.vector.reciprocal(rstd, rstd)
                xh = xpool.tile([P, D], bf16, tag="xh")
                nc.vector.tensor_scalar_mul(xh, xt, rstd)
                tpb = p_pr.tile([P, D], bf16, tag="pp")
                for ic in range(4):
                    nc.tensor.transpose(tpb[:, ic * P:(ic + 1) * P],
                                        xh[:, ic * P:(ic + 1) * P], identity)
                nc.vector.tensor_copy(
                    xhatT[:, :, tt * P:(tt + 1) * P],
                    tpb.rearrange("p (ic q) -> p ic q", ic=4))
                nc.scalar.copy(
                    xhatTb[:, :, tt * P:(tt + 1) * P],
                    tpb.rearrange("p (ic q) -> p ic q", ic=4))
                if idx == 3:
                    for pair in range(4):
                        emit_qproj("l", pair, nc.vector)

            # dilated-branch weights are first needed ~half-way through the
            # local attention phase; the (otherwise idle) gpsimd SWDGE queue
            # issues them so neither HWDGE compute queue pays issue costs.
            for w, br in (("wq", "d"), ("wk", "d"), ("wv", "d"), ("wo", "d")):
                nc.gpsimd.dma_start(w_sb[w, br], wT[w, br][:, :, :])

            # ---- remaining local projections (DVE/ACT casts) ----
            for pair in range(4):
                for half in range(2):
                    emit_kproj("l", pair, half, nc.vector)
            for kc in range(NKC):
                emit_vproj("l", kc, nc.vector)

            # dilated-branch projections become PE filler work interleaved
            # between local attention units (ACT exp binds there, PE has
            # slack); Q/K casts on DVE, V copies on ACT to split the load.
            fillers = []
            for pair in range(4):
                fillers.append(
                    lambda pair=pair: emit_qproj("d", pair, nc.vector))
            for pair in range(4):
                for half in range(2):
                    fillers.append(
                        lambda pair=pair, half=half:
                        emit_kproj("d", pair, half, nc.vector))
            for kc in range(NKC):
                fillers.append(lambda kc=kc: emit_vproj("d", kc, nc.vector))

            # ---- attention ----
            def _masks(br, ex):
                # edge triangle masks: chunk kc==qtile -> tri_lo at
                # q-offset 128*kc; chunk kc==qtile+4 -> tri_hi at
                # 128*(kc-4).  Both strides are uniform in the flat view,
                # so batch into 3 DVE ops instead of 8.
                C = QL if br == "l" else P
                stride = C + P
                hi0 = 4 * C if br == "l" else P
                exf = ex.rearrange("p a b -> p (a b)")
                g1 = exf[:, 0:4 * stride].rearrange(
                    "p (a c) -> p a c", c=stride)[:, :, 0:P]
                # the tri_hi group also has 4 blocks at a uniform stride, but
                # its rearrange view would run past the tensor end, so build
                # the [p, 4, P] access pattern directly (only in-bounds
                # columns are ever touched).
                base = exf[:, hi0:hi0 + P]
                g2 = AP(base.tensor, base.offset,
                        [list(base.ap[0]), [stride, 4], [1, P]])
                for g, tri in ((g1, tri_lo), (g2, tri_hi)):
                    ga, ta = broadcast_tensor_aps(
                        g, tri[:, :].rearrange("p (o b) -> p o b", o=1))
                    nc.vector.tensor_mul(ga, ga, ta)

            def scores_pair(br, pair):
                # the two heads of a pair live on SBUF partition halves
                # 0-63 / 64-127, i.e. row tiles T0/T8 of the 64x128 PE
                # tiling mode -- interleaving their score matmuls lets the
                # PE overlap one tile's LdWeights with the other's Matmul.
                if br == "l":
                    exs = [expp.tile([P, NKC, QL], bf16, tag="exp",
                                     name="ex_e"),
                           expp.tile([P, NKC, QL], bf16, tag="exp",
                                     name="ex_o")]
                    for kc in range(NKC):
                        qlo = max(0, P * (kc - 4))
                        qhi = min(QL, P * kc + P)
                        n = qhi - qlo
                        for sub in range(2):
                            r0 = 64 * sub
                            st = p_st.tile([P, QL], f32, tag="st")
                            nc.tensor.matmul(
                                st[:, :n],
                                KT[br][r0:r0 + 64, pair, kc * P:(kc + 1) * P],
                                QT[br][r0:r0 + 64, pair, qlo:qhi])
                            nc.scalar.activation(
                                exs[sub][:, kc, qlo:qhi], st[:, :n], Exp,
                                scale=1.0 / 32768)
                else:
                    exs = [expp.tile([P, NKC, P], bf16, tag="expd",
                                     name="ex_e"),
                           expp.tile([P, NKC, P], bf16, tag="expd",
                                     name="ex_o")]
                    for half in range(2):
                        for sub in range(2):
                            r0 = 64 * sub
                            st = p_st.tile([P, QL], f32, tag="st")
                            for j in range(4):
                                idx = half * 4 + j
                                rho, s = idx // 2, idx % 2
                                ktv = KT[br][r0:r0 + 64, pair, :].rearrange(
                                    "p (b four) -> p four b", four=DIL
                                )[:, rho, s * P:(s + 1) * P]
                                qtv = QT[br][r0:r0 + 64, pair, :].rearrange(
                                    "p (a four) -> p four a", four=DIL)[:, rho, :]
                                nc.tensor.matmul(st[:, j * P:(j + 1) * P],
                                                 ktv, qtv)
                            nc.scalar.activation(
                                exs[sub][:, half * 4:(half + 1) * 4, :], st,
                                Exp, scale=1.0 / 32768)
                _masks(br, exs[0])
                _masks(br, exs[1])
                return exs

            def av_head(br, h, ex, osb2):
                # all 4 qtiles of this head accumulate into one PSUM bank so
                # the denominators batch: one reciprocal + one broadcast mul.
                chunk_sets = ([(t, range(t, t + 5)) for t in range(NQT)]
                              if br == "l" else
                              [(rho, (rho * 2, rho * 2 + 1)) for rho in range(DIL)])
                op = p_op.tile([P, NQT, DH + 1], f32, tag="op")
                for t, kcs in chunk_sets:
                    kcs = list(kcs)
                    for r, kc in enumerate(kcs):
                        src = (ex[:, kc, t * P:(t + 1) * P] if br == "l"
                               else ex[:, kc, :])
                        nc.tensor.matmul(
                            op[:, t, :], src, V[br][:, kc, h, :],
                            start=(r == 0), stop=(r == len(kcs) - 1))
                rcp = small.tile([P, NQT], f32, tag="rcp")
                nc.vector.reciprocal(
                    rcp, op[:, :, DH:DH + 1].rearrange("p a o -> p (a o)"))
                dst = osb2[:, :, h % 2, :]
                num, rb = broadcast_tensor_aps(
                    op[:, :, 0:DH], rcp.rearrange("p (a o) -> p a o", o=1))
                nc.vector.tensor_mul(dst, num, rb)

            def pair_transpose(br, pair, osb2):
                # two heads' [128q, 64dv] slabs transpose as one [128,128] op
                tpb = p_tr.tile([P, NQT * P], bf16, tag="tpb")
                for t in range(NQT):
                    nc.tensor.transpose(
                        tpb[:, t * P:(t + 1) * P],
                        osb2[:, t].rearrange("p a b -> p (a b)"), identity)
                # d-branch OT copies run in the DVE-paced dilated window --
                # ACT idles there, so it takes them; the final pair's copy
                # gates the tail wo chain, so it goes to the then-idle DVE
                if br == "l" or pair == 3:
                    nc.vector.tensor_copy(OT[br][:, pair, :], tpb)
                else:
                    nc.scalar.copy(OT[br][:, pair, :], tpb)

            def wo_out(br, t):
                ps = p_pr.tile([P, D], f32, tag="pp")
                for pair in range(4):
                    nc.tensor.matmul(
                        ps, OT[br][:, pair, t * P:(t + 1) * P],
                        w_sb["wo", br][:, pair, :],
                        start=(pair == 0), stop=(pair == 3))
                ob = outp.tile([P, D], bf16, tag="ob")
                # local-branch obs land in the dilated window where ACT has
                # slack and DVE is the pacer; dilated obs land in the idle
                # tail where DVE is free
                if br == "l":
                    nc.scalar.copy(ob, ps)
                else:
                    nc.vector.tensor_copy(ob, ps)
                if br == "l":
                    nc.sync.dma_start(out_dram[br][t * P:(t + 1) * P, :], ob)
                else:
                    dst = out_dram[br][:, :].rearrange(
                        "(a four) o -> four a o", four=DIL)[t]
                    nc.scalar.dma_start(dst, ob)

            punits = [("l", p) for p in range(4)] + [("d", p) for p in range(4)]
            pexs = {0: scores_pair(*punits[0])}
            nfill = 0
            for i, (br, pr) in enumerate(punits):
                if i + 1 < len(punits):
                    pexs[i + 1] = scores_pair(*punits[i + 1])
                ex_e, ex_o = pexs.pop(i)
                osb2 = osbp.tile([P, NQT, 2, DH], bf16, tag="osb2",
                                 name="osb2")
                av_head(br, 2 * pr, ex_e, osb2)
                av_head(br, 2 * pr + 1, ex_o, osb2)
                pair_transpose(br, pr, osb2)
                if br == "l":
                    want = min(len(fillers), 5 * (i + 1))
                    while nfill < want:
                        fillers[nfill]()
                        nfill += 1
                if (br, pr) == ("l", 3):
                    while nfill < len(fillers):
                        fillers[nfill]()
                        nfill += 1
                    for t in range(NQT):
                        wo_out("l", t)
            for t in range(NQT):
                wo_out("d", t)

    nc.finalize()
    return nc


def _prep_host(x, key_padding_mask, weights):
    """Build the per-core input maps (weights shared across cores).

    All device tensors are pre-shuffled to the on-chip partition-major layout
    so every DMA moves per-partition-contiguous 2-4KB blocks.
    """
    x = np.asarray(x, dtype=np.float32).reshape(L, D)
    kpm = np.asarray(key_padding_mask).reshape(L).astype(bool)

    shared = {}
    for name, arr in weights.items():
        if name.startswith("wq") or name.startswith("wk"):
            pre = arr.reshape(4, P, D).transpose(1, 0, 2)
            shared[name] = np.ascontiguousarray(pre).astype(FP8)
        else:
            pre = arr.reshape(4, P, D).transpose(1, 0, 2)
            shared[name] = np.ascontiguousarray(pre).astype(BF16)

    idx = np.arange(P)
    tri_lo = (idx[:, None] >= idx[None, :]).astype(BF16)
    tri_hi = (idx[:, None] <= idx[None, :]).astype(BF16)
    shared["tri_lo"], shared["tri_hi"] = tri_lo, tri_hi

    valid_full = np.zeros(L + 2 * WIN, dtype=np.float32)
    valid_full[WIN:WIN + L] = (~kpm).astype(np.float32)

    in_maps = []
    for c in range(NCORES):
        lo = c * QL - WIN
        xnc = np.zeros((KL, D), dtype=np.float32)
        a, b = max(lo, 0), min(lo + KL, L)
        xnc[a - lo:b - lo] = x[a:b]
        v = valid_full[lo + WIN:lo + WIN + KL]  # validity of keys lo..lo+KL
        cm_l = v.reshape(NKC, P).T.astype(np.float32)
        # dilated chunk idx = rho*2+s holds keys lk = 4*(128*s + p) + rho
        cm_d = np.empty((P, NKC), dtype=np.float32)
        for rho in range(DIL):
            for s in range(2):
                lk = DIL * (P * s + idx) + rho
                cm_d[:, rho * 2 + s] = v[lk]
        m = dict(shared)
        m["xn"] = np.ascontiguousarray(
            xnc.reshape(NKC, P, D).transpose(1, 0, 2)).astype(BF16)
        m["colmask_l"] = np.ascontiguousarray(cm_l)
        m["colmask_d"] = np.ascontiguousarray(cm_d)
        in_maps.append(m)
    return in_maps


def kernel(x, key_padding_mask, wq_l, wk_l, wv_l, wo_l,
           wq_d, wk_d, wv_d, wo_d, g_q, g_kv, **run_kwargs):
    from concourse.bass_utils import run_bass_kernel_spmd

    g_q = np.asarray(g_q, dtype=np.float32)
    g_kv = np.asarray(g_kv, dtype=np.float32)
    scale = 1.0 / np.sqrt(DH)
    weights = {
        "wqT_l": np.asarray(wq_l, np.float32).T * (g_q * scale)[:, None] * 512.0,
        "wkT_l": np.asarray(wk_l, np.float32).T * g_kv[:, None] * 64.0,
        "wvT_l": np.asarray(wv_l, np.float32).T * g_kv[:, None],
        "woT_l": np.asarray(wo_l, np.float32).T,
        "wqT_d": np.asarray(wq_d, np.float32).T * (g_q * scale)[:, None] * 512.0,
        "wkT_d": np.asarray(wk_d, np.float32).T * g_kv[:, None] * 64.0,
        "wvT_d": np.asarray(wv_d, np.float32).T * g_kv[:, None],
        "woT_d": np.asarray(wo_d, np.float32).T,
    }
    in_maps = _prep_host(x, key_padding_mask, weights)

    if "nc" not in _STATE:
        _STATE["nc"] = _build_nc()
    res = run_bass_kernel_spmd(_STATE["nc"], in_maps,
                               core_ids=list(range(NCORES)), **run_kwargs)
    _STATE["last_result"] = res

    out_l = np.concatenate(
        [np.asarray(res.results[c]["out_l"], dtype=np.float32)
         for c in range(NCORES)], axis=0).reshape(1, L, D)
    out_d = np.concatenate(
        [np.asarray(res.results[c]["out_d"], dtype=np.float32)
         for c in range(NCORES)], axis=0).reshape(1, L, D)
    return (out_l, out_d)


# revision 60
# speedup vs baseline: 1.0212x; 1.0160x over previous
"""Trainium2 Bass kernel for dual-branch local+dilated windowed attention.

Problem: B=1, L=4096, D=512, H=8 heads (dh=64), window=+-256, dilation=4.
reference returns (out_local, out_dilated), each [1, L, D] fp32.

Sharding: sequence (L) sharded across 8 cores; each core owns 512 query rows
and loads a 1024-row key slice (256-row halo each side, zero-padded at the
sequence edges).  All weights are replicated, pre-transposed, pre-shuffled to
the on-chip partition-major layout host-side (so every DMA is per-partition
contiguous with large descriptors), and cast to bf16 (wq/wk to fp8) with the
rmsnorm gains and the 1/sqrt(dh) score scale folded in.

v2 structure (vs the v1 baseline at ~131us):
  - DMA: x and weights pre-shuffled host-side to [128, ...] partition-major
    so descriptors are 2-4KB contiguous; spread across the sync/scalar/gpsimd
    queues in first-needed order; x split in 4 chunks so rmsnorm starts early.
  - attention pipelined over 16 (branch, head) units with scores emitted one
    unit ahead, 3 score-psum banks to decouple PE from ACT exp, and the
    dilated branch's projections emitted as PE filler work between local
    units (ACT exp is the per-unit bottleneck for the local branch).
  - per-head AV accumulates all 4 query tiles into one PSUM bank so the
    softmax denominators batch into a single reciprocal + one broadcast mul.
  - AV output transposes are head-paired: two heads' [128,64] normalized
    context slabs transpose as one [128,128] PE op straight into the OT
    layout that the (row-parallel) Wo matmul wants.
  - PSUM->SBUF copies balanced across DVE/ACT/GPSIMD by phase load.
"""

import numpy as np
import ml_dtypes

L, D, H, DH = 4096, 512, 8, 64
WIN, DIL = 256, 4
EPS = 1e-6
NCORES = 8
QL = L // NCORES          # 512 queries per core
KL = QL + 2 * WIN         # 1024 keys per core (halo)
P = 128
NKC = KL // P             # 8 key chunks
NQT = QL // P             # 4 query tiles
BF16 = ml_dtypes.bfloat16
FP8 = ml_dtypes.float8_e4m3fn

_STATE = {}


def _build_nc():
    import concourse.bacc as bacc
    import concourse.tile as tile
    import concourse.mybir as mybir
    from concourse.masks import make_identity
    from concourse.bass import broadcast_tensor_aps, AP

    f32 = mybir.dt.float32
    bf16 = mybir.dt.bfloat16
    fp8 = mybir.dt.float8e4
    Exp = mybir.ActivationFunctionType.Exp
    Square = mybir.ActivationFunctionType.Square
    Sqrt = mybir.ActivationFunctionType.Sqrt
    DR = mybir.MatmulPerfMode.DoubleRow
    Mult = mybir.AluOpType.mult

    nc = bacc.Bacc()

    xn = nc.dram_tensor("xn", [P, NKC, D], bf16, kind="ExternalInput")
    wT = {}
    for br in ("l", "d"):
        for w in ("wq", "wk", "wv", "wo"):
            dt_w = fp8 if w in ("wq", "wk") else bf16
            wT[w, br] = nc.dram_tensor(f"{w}T_{br}", [P, 4, D], dt_w,
                                       kind="ExternalInput")
    tri_lo_d = nc.dram_tensor("tri_lo", [P, P], bf16, kind="ExternalInput")
    tri_hi_d = nc.dram_tensor("tri_hi", [P, P], bf16, kind="ExternalInput")
    colmask_d_ = {
        "l": nc.dram_tensor("colmask_l", [P, NKC], f32, kind="ExternalInput"),
        "d": nc.dram_tensor("colmask_d", [P, NKC], f32, kind="ExternalInput"),
    }
    out_dram = {
        "l": nc.dram_tensor("out_l", [QL, D], bf16, kind="ExternalOutput"),
        "d": nc.dram_tensor("out_d", [QL, D], bf16, kind="ExternalOutput"),
    }

    with tile.TileContext(nc) as tc:
        with (
            tc.tile_pool(name="singles", bufs=1) as singles,
            tc.tile_pool(name="xpool", bufs=3) as xpool,
            tc.tile_pool(name="small", bufs=8) as small,
            tc.tile_pool(name="expp", bufs=4) as expp,
            tc.tile_pool(name="osbp", bufs=2) as osbp,
            tc.tile_pool(name="outp", bufs=2) as outp,
            tc.tile_pool(name="pst", bufs=3, space="PSUM") as p_st,
            tc.tile_pool(name="pop", bufs=2, space="PSUM") as p_op,
            tc.tile_pool(name="ppr", bufs=2, space="PSUM") as p_pr,
            tc.tile_pool(name="ptr", bufs=1, space="PSUM") as p_tr,
        ):
            identity = singles.tile([P, P], bf16)
            make_identity(nc, identity)
            eps_t = singles.tile([P, 1], f32, name="eps")
            nc.vector.memset(eps_t, EPS)

            # ---- input + weight DMAs: partition-major contiguous layouts,
            # spread over the two HWDGE queues + SWDGE, first-needed first.
            # query-range tiles (tt 2-5) first so the Q projection can start
            # before the halo tiles land
            xsb = singles.tile([P, NKC, D], bf16, name="xsb")
            for eng, t0, ntt in ((nc.sync, 2, 2), (nc.scalar, 4, 2),
                                 (nc.sync, 0, 2), (nc.scalar, 6, 2)):
                eng.dma_start(xsb[:, t0:t0 + ntt, :], xn[:, t0:t0 + ntt, :])
            w_sb = {}
            for w, br in [("wq", "l"), ("wk", "l"), ("wv", "l"), ("wo", "l"),
                          ("wq", "d"), ("wk", "d"), ("wv", "d"), ("wo", "d")]:
                w_sb[w, br] = singles.tile(
                    [P, 4, D], fp8 if w in ("wq", "wk") else bf16,
                    name=f"{w}_{br}")
            for w, br in (("wq", "l"), ("wk", "l"), ("wv", "l"), ("wo", "l")):
                nc.sync.dma_start(w_sb[w, br], wT[w, br][:, :, :])
            colmask = {}
            for br in ("l", "d"):
                colmask[br] = singles.tile([P, NKC], f32, name=f"cm_{br}")
                nc.gpsimd.dma_start(colmask[br], colmask_d_[br][:, :])
            tri_lo = singles.tile([P, P], bf16)
            nc.gpsimd.dma_start(tri_lo, tri_lo_d[:, :])
            tri_hi = singles.tile([P, P], bf16)
            nc.gpsimd.dma_start(tri_hi, tri_hi_d[:, :])

            xhatT = singles.tile([P, 4, KL], fp8, name="xhatT")
            xhatTb = singles.tile([P, 4, KL], bf16, name="xhatTb")



            QT, KT, V, OT = {}, {}, {}, {}
            for br in ("l", "d"):
                QT[br] = singles.tile([P, 4, QL], bf16, name=f"QT_{br}")
                KT[br] = singles.tile([P, 4, KL], bf16, name=f"KT_{br}")
                V[br] = singles.tile([P, NKC, H, DH + 1], bf16, name=f"V_{br}")
                OT[br] = singles.tile([P, 4, QL], bf16, name=f"OT_{br}")

            def key_cols_ap(ic, kc, br):
                # lhsT [128, 128] of xhat^T columns for key chunk kc
                if br == "l":
                    return xhatTb[:, ic, kc * P:(kc + 1) * P]
                rho, s = kc // 2, kc % 2
                return xhatTb[:, ic, :].rearrange(
                    "p (b four) -> p four b", four=DIL)[:, rho, s * P:(s + 1) * P]

            # ---- projection emitters (engine-parameterized copies) ----
            # NOTE: GPSIMD cannot read PSUM, so all PSUM->SBUF casts go on
            # DVE/ACT; GPSIMD takes the SBUF-only colmask multiplies.
            def _copy(eng, dst, src):
                if eng is nc.scalar:
                    nc.scalar.copy(dst, src)
                else:
                    eng.tensor_copy(dst, src)

            def emit_qproj(br, pair, ceng):
                ps = p_pr.tile([P, D], f32, tag="pp")
                for ic in (0, 2):
                    nc.tensor.matmul(
                        ps, w_sb["wq", br][:, ic:ic + 2, pair * P:(pair + 1) * P],
                        xhatT[:, ic:ic + 2, WIN:WIN + QL],
                        start=(ic == 0), stop=(ic == 2), perf_mode=DR)
                _copy(ceng, QT[br][:, pair, :], ps)

            def emit_kproj(br, pair, half, ceng):
                ps = p_pr.tile([P, D], f32, tag="pp")
                for ic in (0, 2):
                    nc.tensor.matmul(
                        ps, w_sb["wk", br][:, ic:ic + 2, pair * P:(pair + 1) * P],
                        xhatT[:, ic:ic + 2, half * D:(half + 1) * D],
                        start=(ic == 0), stop=(ic == 2), perf_mode=DR)
                _copy(ceng, KT[br][:, pair, half * D:(half + 1) * D], ps)

            def emit_vproj(br, kc, ceng):
                ps = p_pr.tile([P, D], f32, tag="pp")
                for ic in range(4):
                    nc.tensor.matmul(
                        ps, key_cols_ap(ic, kc, br),
                        w_sb["wv", br][:, ic, :],
                        start=(ic == 0), stop=(ic == 3))
                _copy(ceng, V[br][:, kc, :, 0:DH],
                      ps.rearrange("p (h dv) -> p h dv", h=H))
                nc.vector.memset(V[br][:, kc, :, DH:DH + 1], 1.0)
                nc.vector.tensor_scalar_mul(
                    V[br][:, kc], V[br][:, kc], colmask[br][:, kc:kc + 1])

            # ---- rmsnorm + transpose (x^2 sums alternate ACT/DVE) ----
            # query-range tiles first; the local Q projection is emitted as
            # soon as its four token tiles (tt 2-5) are transposed.
            for idx, tt in enumerate((2, 3, 4, 5, 0, 1, 6, 7)):
                xt = xsb[:, tt, :]
                sqd = xpool.tile([P, D], bf16, tag="sqd")
                ssum = small.tile([P, 1], f32, tag="ssum")
                if tt % 2 == 0:
                    nc.scalar.activation(sqd, xt, Square, accum_out=ssum)
                else:
                    nc.vector.scalar_tensor_tensor(
                        sqd, xt, 1.0, xt, Mult, Mult, accum_out=ssum)
                rstd = small.tile([P, 1], f32, tag="rstd")
                nc.scalar.activation(rstd, ssum, Sqrt, bias=eps_t, scale=1.0 / D)
                nc.vector.reciprocal(rstd, rstd)
                xh = xpool.tile([P, D], bf16, tag="xh")
                nc.vector.tensor_scalar_mul(xh, xt, rstd)
                tpb = p_pr.tile([P, D], bf16, tag="pp")
                for ic in range(4):
                    nc.tensor.transpose(tpb[:, ic * P:(ic + 1) * P],
                                        xh[:, ic * P:(ic + 1) * P], identity)
                nc.vector.tensor_copy(
                    xhatT[:, :, tt * P:(tt + 1) * P],
                    tpb.rearrange("p (ic q) -> p ic q", ic=4))
                nc.scalar.copy(
                    xhatTb[:, :, tt * P:(tt + 1) * P],
                    tpb.rearrange("p (ic q) -> p ic q", ic=4))
                if idx == 3:
                    for pair in range(4):
                        emit_qproj("l", pair, nc.vector)

            # dilated-branch weights are first needed ~half-way through the
            # local attention phase; the (otherwise idle) gpsimd SWDGE queue
            # issues them so neither HWDGE compute queue pays issue costs.
            for w, br in (("wq", "d"), ("wk", "d"), ("wv", "d"), ("wo", "d")):
                nc.gpsimd.dma_start(w_sb[w, br], wT[w, br][:, :, :])

            # ---- remaining local projections (DVE/ACT casts) ----
            for pair in range(4):
                for half in range(2):
                    emit_kproj("l", pair, half, nc.vector)
            for kc in range(NKC):
                emit_vproj("l", kc, nc.scalar)

            # dilated-branch projections become PE filler work interleaved
            # between local attention units (ACT exp binds there, PE has
            # slack); Q/K casts on DVE, V copies on ACT to split the load.
            fillers = []
            for pair in range(4):
                fillers.append(
                    lambda pair=pair: emit_qproj("d", pair, nc.vector))
            for pair in range(4):
                for half in range(2):
                    fillers.append(
                        lambda pair=pair, half=half:
                        emit_kproj("d", pair, half, nc.vector))
            for kc in range(NKC):
                fillers.append(lambda kc=kc: emit_vproj("d", kc, nc.vector))

            # ---- attention ----
            def _masks(br, ex):
                # edge triangle masks: chunk kc==qtile -> tri_lo at
                # q-offset 128*kc; chunk kc==qtile+4 -> tri_hi at
                # 128*(kc-4).  Both strides are uniform in the flat view,
                # so batch into 3 DVE ops instead of 8.
                C = QL if br == "l" else P
                stride = C + P
                hi0 = 4 * C if br == "l" else P
                exf = ex.rearrange("p a b -> p (a b)")
                g1 = exf[:, 0:4 * stride].rearrange(
                    "p (a c) -> p a c", c=stride)[:, :, 0:P]
                # the tri_hi group also has 4 blocks at a uniform stride, but
                # its rearrange view would run past the tensor end, so build
                # the [p, 4, P] access pattern directly (only in-bounds
                # columns are ever touched).
                base = exf[:, hi0:hi0 + P]
                g2 = AP(base.tensor, base.offset,
                        [list(base.ap[0]), [stride, 4], [1, P]])
                for g, tri in ((g1, tri_lo), (g2, tri_hi)):
                    ga, ta = broadcast_tensor_aps(
                        g, tri[:, :].rearrange("p (o b) -> p o b", o=1))
                    nc.vector.tensor_mul(ga, ga, ta)

            def scores_pair(br, pair):
                # the two heads of a pair live on SBUF partition halves
                # 0-63 / 64-127, i.e. row tiles T0/T8 of the 64x128 PE
                # tiling mode -- interleaving their score matmuls lets the
                # PE overlap one tile's LdWeights with the other's Matmul.
                if br == "l":
                    exs = [expp.tile([P, NKC, QL], bf16, tag="exp",
                                     name="ex_e"),
                           expp.tile([P, NKC, QL], bf16, tag="exp",
                                     name="ex_o")]
                    for kc in range(NKC):
                        qlo = max(0, P * (kc - 4))
                        qhi = min(QL, P * kc + P)
                        n = qhi - qlo
                        for sub in range(2):
                            r0 = 64 * sub
                            st = p_st.tile([P, QL], f32, tag="st")
                            nc.tensor.matmul(
                                st[:, :n],
                                KT[br][r0:r0 + 64, pair, kc * P:(kc + 1) * P],
                                QT[br][r0:r0 + 64, pair, qlo:qhi])
                            nc.scalar.activation(
                                exs[sub][:, kc, qlo:qhi], st[:, :n], Exp,
                                scale=1.0 / 32768)
                else:
                    exs = [expp.tile([P, NKC, P], bf16, tag="expd",
                                     name="ex_e"),
                           expp.tile([P, NKC, P], bf16, tag="expd",
                                     name="ex_o")]
                    for half in range(2):
                        sts = [p_st.tile([P, QL], f32, tag="st", name="st")
                               for _ in range(2)]
                        # alternate the two heads' (row-tile T0/T8) matmuls
                        # per j so LdWeights of one tile overlaps the other
                        # tile's Matmul
                        for j in range(4):
                            idx = half * 4 + j
                            rho, s = idx // 2, idx % 2
                            for sub in range(2):
                                r0 = 64 * sub
                                ktv = KT[br][r0:r0 + 64, pair, :].rearrange(
                                    "p (b four) -> p four b", four=DIL
                                )[:, rho, s * P:(s + 1) * P]
                                qtv = QT[br][r0:r0 + 64, pair, :].rearrange(
                                    "p (a four) -> p four a", four=DIL)[:, rho, :]
                                nc.tensor.matmul(sts[sub][:, j * P:(j + 1) * P],
                                                 ktv, qtv)
                        for sub in range(2):
                            nc.scalar.activation(
                                exs[sub][:, half * 4:(half + 1) * 4, :],
                                sts[sub], Exp, scale=1.0 / 32768)
                _masks(br, exs[0])
                _masks(br, exs[1])
                return exs

            def av_head(br, h, ex, osb2):
                # all 4 qtiles of this head accumulate into one PSUM bank so
                # the denominators batch: one reciprocal + one broadcast mul.
                chunk_sets = ([(t, range(t, t + 5)) for t in range(NQT)]
                              if br == "l" else
                              [(rho, (rho * 2, rho * 2 + 1)) for rho in range(DIL)])
                op = p_op.tile([P, NQT, DH + 1], f32, tag="op")
                for t, kcs in chunk_sets:
                    kcs = list(kcs)
                    for r, kc in enumerate(kcs):
                        src = (ex[:, kc, t * P:(t + 1) * P] if br == "l"
                               else ex[:, kc, :])
                        nc.tensor.matmul(
                            op[:, t, :], src, V[br][:, kc, h, :],
                            start=(r == 0), stop=(r == len(kcs) - 1))
                rcp = small.tile([P, NQT], f32, tag="rcp")
                nc.vector.reciprocal(
                    rcp, op[:, :, DH:DH + 1].rearrange("p a o -> p (a o)"))
                dst = osb2[:, :, h % 2, :]
                num, rb = broadcast_tensor_aps(
                    op[:, :, 0:DH], rcp.rearrange("p (a o) -> p a o", o=1))
                nc.vector.tensor_mul(dst, num, rb)

            def pair_transpose(br, pair, osb2):
                # two heads' [128q, 64dv] slabs transpose as one [128,128] op
                tpb = p_tr.tile([P, NQT * P], bf16, tag="tpb")
                for t in range(NQT):
                    nc.tensor.transpose(
                        tpb[:, t * P:(t + 1) * P],
                        osb2[:, t].rearrange("p a b -> p (a b)"), identity)
                # d-branch OT copies run in the DVE-paced dilated window --
                # ACT idles there, so it takes them
                if br == "l":
                    nc.vector.tensor_copy(OT[br][:, pair, :], tpb)
                else:
                    nc.scalar.copy(OT[br][:, pair, :], tpb)

            def wo_out(br, t):
                ps = p_pr.tile([P, D], f32, tag="pp")
                for pair in range(4):
                    nc.tensor.matmul(
                        ps, OT[br][:, pair, t * P:(t + 1) * P],
                        w_sb["wo", br][:, pair, :],
                        start=(pair == 0), stop=(pair == 3))
                ob = outp.tile([P, D], bf16, tag="ob")
                # local-branch obs land in the dilated window where ACT has
                # slack and DVE is the pacer; dilated obs land in the idle
                # tail where DVE is free
                if br == "l":
                    nc.scalar.copy(ob, ps)
                else:
                    nc.vector.tensor_copy(ob, ps)
                if br == "l":
                    nc.sync.dma_start(out_dram[br][t * P:(t + 1) * P, :], ob)
                else:
                    dst = out_dram[br][:, :].rearrange(
                        "(a four) o -> four a o", four=DIL)[t]
                    nc.scalar.dma_start(dst, ob)

            punits = [("l", p) for p in range(4)] + [("d", p) for p in range(4)]
            pexs = {0: scores_pair(*punits[0])}
            nfill = 0
            for i, (br, pr) in enumerate(punits):
                if i + 1 < len(punits):
                    pexs[i + 1] = scores_pair(*punits[i + 1])
                ex_e, ex_o = pexs.pop(i)
                osb2 = osbp.tile([P, NQT, 2, DH], bf16, tag="osb2",
                                 name="osb2")
                av_head(br, 2 * pr, ex_e, osb2)
                av_head(br, 2 * pr + 1, ex_o, osb2)
                pair_transpose(br, pr, osb2)
                if br == "l":
                    want = min(len(fillers), 5 * (i + 1))
                    while nfill < want:
                        fillers[nfill]()
                        nfill += 1
                if (br, pr) == ("l", 3):
                    while nfill < len(fillers):
                        fillers[nfill]()
                        nfill += 1
                    for t in range(NQT):
                        wo_out("l", t)
            for t in range(NQT):
                wo_out("d", t)

    nc.finalize()
    return nc


def _prep_host(x, key_padding_mask, weights):
    """Build the per-core input maps (weights shared across cores).

    All device tensors are pre-shuffled to the on-chip partition-major layout
    so every DMA moves per-partition-contiguous 2-4KB blocks.
    """
    x = np.asarray(x, dtype=np.float32).reshape(L, D)
    kpm = np.asarray(key_padding_mask).reshape(L).astype(bool)

    shared = {}
    for name, arr in weights.items():
        if name.startswith("wq") or name.startswith("wk"):
            pre = arr.reshape(4, P, D).transpose(1, 0, 2)
            shared[name] = np.ascontiguousarray(pre).astype(FP8)
        else:
            pre = arr.reshape(4, P, D).transpose(1, 0, 2)
            shared[name] = np.ascontiguousarray(pre).astype(BF16)

    idx = np.arange(P)
    tri_lo = (idx[:, None] >= idx[None, :]).astype(BF16)
    tri_hi = (idx[:, None] <= idx[None, :]).astype(BF16)
    shared["tri_lo"], shared["tri_hi"] = tri_lo, tri_hi

    valid_full = np.zeros(L + 2 * WIN, dtype=np.float32)
    valid_full[WIN:WIN + L] = (~kpm).astype(np.float32)

    in_maps = []
    for c in range(NCORES):
        lo = c * QL - WIN
        xnc = np.zeros((KL, D), dtype=np.float32)
        a, b = max(lo, 0), min(lo + KL, L)
        xnc[a - lo:b - lo] = x[a:b]
        v = valid_full[lo + WIN:lo + WIN + KL]  # validity of keys lo..lo+KL
        cm_l = v.reshape(NKC, P).T.astype(np.float32)
        # dilated chunk idx = rho*2+s holds keys lk = 4*(128*s + p) + rho
        cm_d = np.empty((P, NKC), dtype=np.float32)
        for rho in range(DIL):
            for s in range(2):
                lk = DIL * (P * s + idx) + rho
                cm_d[:, rho * 2 + s] = v[lk]
        m = dict(shared)
        m["xn"] = np.ascontiguousarray(
            xnc.reshape(NKC, P, D).transpose(1, 0, 2)).astype(BF16)
        m["colmask_l"] = np.ascontiguousarray(cm_l)
        m["colmask_d"] = np.ascontiguousarray(cm_d)
        in_maps.append(m)
    return in_maps


def kernel(x, key_padding_mask, wq_l, wk_l, wv_l, wo_l,
           wq_d, wk_d, wv_d, wo_d, g_q, g_kv, **run_kwargs):
    from concourse.bass_utils import run_bass_kernel_spmd

    g_q = np.asarray(g_q, dtype=np.float32)
    g_kv = np.asarray(g_kv, dtype=np.float32)
    scale = 1.0 / np.sqrt(DH)
    weights = {
        "wqT_l": np.asarray(wq_l, np.float32).T * (g_q * scale)[:, None] * 512.0,
        "wkT_l": np.asarray(wk_l, np.float32).T * g_kv[:, None] * 64.0,
        "wvT_l": np.asarray(wv_l, np.float32).T * g_kv[:, None],
        "woT_l": np.asarray(wo_l, np.float32).T,
        "wqT_d": np.asarray(wq_d, np.float32).T * (g_q * scale)[:, None] * 512.0,
        "wkT_d": np.asarray(wk_d, np.float32).T * g_kv[:, None] * 64.0,
        "wvT_d": np.asarray(wv_d, np.float32).T * g_kv[:, None],
        "woT_d": np.asarray(wo_d, np.float32).T,
    }
    in_maps = _prep_host(x, key_padding_mask, weights)

    if "nc" not in _STATE:
        _STATE["nc"] = _build_nc()
    res = run_bass_kernel_spmd(_STATE["nc"], in_maps,
                               core_ids=list(range(NCORES)), **run_kwargs)
    _STATE["last_result"] = res

    out_l = np.concatenate(
        [np.asarray(res.results[c]["out_l"], dtype=np.float32)
         for c in range(NCORES)], axis=0).reshape(1, L, D)
    out_d = np.concatenate(
        [np.asarray(res.results[c]["out_d"], dtype=np.float32)
         for c in range(NCORES)], axis=0).reshape(1, L, D)
    return (out_l, out_d)
